# revision 34
# baseline (speedup 1.0000x reference)
"""Windowed 3D attention (nn_Attention3d) Trainium2 kernel, 8-core SPMD.

Sharding: the 8x8 grid of 16x16 spatial windows is split by row across the
8 NeuronCores. Each core processes a (C=256, F=2, 16, W=128) slab of x:
8 windows x 8 heads of independent 512-token attention, plus the QKV and
output projections for its slab. Host-side numpy does the window
permutation and weight transposes; cores see dense (C, tokens) panels.

Kernel structure (per window, fp32r matmuls, fp32 accumulation):
- Q/K projected inner-major (lhsT = wqk^T chunks), V projected token-major
  (lhsT = x panel), so attention needs no transposes anywhere:
  simT[j,i] = K^T-slice^T @ Q^T directly, with the two heads of an inner
  chunk row-packed into complementary 64-partition PE row groups writing
  one 2-bank PSUM tile.
- U = exp(simT/8) on ACT (one [128,1024] activation per head pair).
- O^T = (V|1)^T @ U accumulated over j; the appended ones column yields the
  softmax denominators s as PSUM row 64 for free. Normalization is batched
  per window: each O-group drains to SBUF with one [65,T] DVE copy (frees
  the po PSUM WAR in <1us), the 8 denominator rows hop to one [8,T] tile
  via SBUF->SBUF DMAs, and a single 8-lane DVE reciprocal covers the whole
  window (a [1,T] reciprocal is one-lane serial, ~4.3us each -- eight of
  them made DVE the kernel bottleneck). Broadcast (GPSIMD) + multiply (DVE)
  run in the window tail, overlapped with the next window's projections.
- Output projection consumes O^T chunks as lhsT; two i-tiles pack into one
  PSUM bank (from proj_ps, not o_ps, keeping the O stage's PSUM rotation
  free of y-stage consumers); bias added on DVE; strided output stores ride
  the ACT hardware-DGE queue so the SP queue never head-of-line blocks x
  prefetches behind stores.
- The output projection is software-pipelined one window behind (emitted at
  pair 1 so the previous normalize tail has two phases of slack); x panels
  prefetch two windows ahead on the sync DMA queue.
- Timing repeats run inside a tc.For_i hardware loop: NEFF size is constant
  in the repeat count, so repeat-differencing isolates true steady-state
  execution from NEFF-size-proportional dispatch overhead.

Measured (8 NeuronCores, axon, repeat-differenced): ~273 us per iteration,
rel err ~3e-4 vs the fp32 reference (fp32r matmul rounding).
"""

import os
import sys

sys.path.insert(0, "/opt/trn_rl_repo")
os.environ.setdefault("MYCRO_LOCAL_CACHE", "1")

import numpy as np
from contextlib import ExitStack

import concourse.bass as bass  # noqa: F401  (AP types)
import concourse.bacc as bacc
from concourse import mybir, tile
from concourse.alu_op_type import AluOpType
from concourse.bass_utils import run_bass_kernel_spmd

FP = mybir.dt.float32
HEADS = 8
DH = 64
WS = 16
C = 256
INNER = 512  # HEADS * DH
F = 2
H = 128
W = 128
NW = 8  # windows per core (one window-grid row)
T = F * WS * WS  # 512 tokens per window
NPIX = NW * T  # 4096 pixels per core slab
XK = 2  # C=256 -> 2 partition chunks of 128
N_CORES = 8
SCALE = DH ** -0.5
MR = mybir.dt.float32r  # rounded fp32: 1.5 vs 2.0 PE cyc/row; producers must emit it
BF16_ATT = os.environ.get("BF16_ATT", "0") == "1"  # bf16 sim/O matmul operands
BA = mybir.dt.bfloat16 if BF16_ATT else MR


def _r(ap):
    return ap

_CACHE = {}


def _build(repeat=1):
    nc = bacc.Bacc("TRN2", target_bir_lowering=False, debug=False)

    xw = nc.dram_tensor("xw", [C, NPIX], MR, kind="ExternalInput").ap()
    wqkT = nc.dram_tensor("wqkT", [C, 2 * INNER], MR, kind="ExternalInput").ap()
    wvT = nc.dram_tensor("wvT", [C, INNER], MR, kind="ExternalInput").ap()
    woT = nc.dram_tensor("woT", [INNER, C], MR, kind="ExternalInput").ap()
    bo = nc.dram_tensor("bo", [1, C], FP, kind="ExternalInput").ap()
    out = nc.dram_tensor("out", [NW, T, C], FP, kind="ExternalOutput").ap()

    Exp = mybir.ActivationFunctionType.Exp

    with tile.TileContext(nc) as tc, ExitStack() as ctx:
        def pool(name, bufs, space="SBUF"):
            return ctx.enter_context(tc.tile_pool(name=name, bufs=bufs, space=space))

        consts = pool("consts", 1)
        xpool = pool("x", 3)
        qkpool = pool("qk", 2)
        vpool = pool("v", 2)
        upool = pool("u", 2)
        rpool = pool("r", 2)
        rbpool = pool("rb", 1)  # one [64,8T] broadcast + its packed row
        orpool = pool("oraw", 2)
        opool = pool("o", 2)
        ypool = pool("y", 2)

        proj_ps = pool("proj_ps", 2, space="PSUM")
        sim_ps = pool("sim_ps", 2, space="PSUM")
        o_ps = pool("o_ps", 2, space="PSUM")
        y_ps = o_ps  # y tiles borrow o_ps slots; proj pool stays open for the next window

        def load_x(yy):
            ts = []
            for k in range(XK):
                t_ = xpool.tile([128, T], MR, tag=f"x{k}", name=f"x{k}")
                nc.sync.dma_start(t_[:, :], xw[k * 128:(k + 1) * 128, yy * T:(yy + 1) * T])
                ts.append(t_)
            return ts

        def emit_y(yy, o_sb):
            # output projection + bias (pipelined one window behind so the
            # PE stream interleaves it with the NEXT window's projections).
            # Two i-tiles pack into one PSUM bank (N=256 halves). py comes
            # from proj_ps, NOT o_ps: sharing o_ps with po made the next
            # window's O matmuls WAR-wait on the y-stage DVE consumer
            # (-84us/repeat). The store rides the ACT hardware DGE queue so
            # the SP queue never head-of-line blocks x prefetches behind a
            # store whose ysb data isn't ready yet (-75us/repeat).
            for it2 in range(2):
                py = proj_ps.tile([128, 2 * C], FP, tag="projy", name="py")
                for half in range(2):
                    it = 2 * it2 + half
                    for m in range(4):
                        nc.tensor.matmul(
                            py[:, half * C:(half + 1) * C],
                            lhsT=_r(o_sb[m][:, it * 128:(it + 1) * 128]),
                            rhs=_r(wo_sb[m][:, :]),
                            start=(m == 0),
                            stop=(m == 3),
                        )
                ysb = ypool.tile([128, 2 * C], FP, tag="y", name="ysb")
                nc.vector.tensor_tensor(
                    ysb[:, :], py[:, :], bias2_bc[:, :], AluOpType.add
                )
                dst = out[yy, it2 * 256:(it2 + 1) * 256, :].rearrange(
                    "(h p) c -> p h c", h=2
                )
                nc.scalar.dma_start(dst, ysb[:, :].rearrange("p (h c) -> p h c", h=2))

        # pin the exp activation-table set while the initial DMAs stream,
        # so the ~2.7us ACT_TABLE_LOAD is off window 0's critical path
        warm = consts.tile([128, 8], FP, tag="warm", name="warm")
        nc.vector.memset(warm[:, :], 0.0)
        nc.scalar.activation(warm[:, :], warm[:, :], Exp, scale=1.0)
        # HAM warmup: dummy matmuls during the initial DMA wait so the PE
        # clock-gate is at 2.4GHz (not cold 1.2) when window 0 starts
        wmm = consts.tile([128, 640], MR, tag="wmm", name="wmm")
        nc.vector.memset(wmm[:, :].bitcast(FP), 0.0)
        for _ in range(26):
            wps = proj_ps.tile([128, T], FP, tag="projy", name="wps")
            nc.tensor.matmul(
                wps[:, :], lhsT=wmm[:, 0:128], rhs=wmm[:, 128:640],
                start=True, stop=True,
            )

        # ---- weights / constants (loaded once) ----
        wqk_sb = []
        wv_sb = []
        wo_sb = []
        for k in range(XK):
            t_ = consts.tile([128, 2 * INNER], MR, tag=f"wqk{k}", name=f"wqk{k}")
            nc.sync.dma_start(t_[:, :], wqkT[k * 128:(k + 1) * 128, :])
            wqk_sb.append(t_)
            t_ = consts.tile([128, INNER], MR, tag=f"wv{k}", name=f"wv{k}")
            nc.sync.dma_start(t_[:, :], wvT[k * 128:(k + 1) * 128, :])
            wv_sb.append(t_)
        for m in range(4):
            t_ = consts.tile([128, C], MR, tag=f"wo{m}", name=f"wo{m}")
            nc.sync.dma_start(t_[:, :], woT[m * 128:(m + 1) * 128, :])
            wo_sb.append(t_)
        bo_sb = consts.tile([1, C], FP, tag="bo")
        nc.sync.dma_start(bo_sb[:, :], bo[:, :])
        bias_bc = consts.tile([128, C], FP, tag="bias_bc")
        nc.gpsimd.partition_broadcast(bias_bc[:, :], bo_sb[:, :])
        bias2_bc = consts.tile([128, 2 * C], FP, tag="bias2_bc")
        nc.vector.tensor_copy(bias2_bc[:, 0:C], bias_bc[:, :])
        nc.vector.tensor_copy(bias2_bc[:, C:2 * C], bias_bc[:, :])
        ones8 = consts.tile([128, HEADS], BA, tag="ones8")
        nc.scalar.activation(
            ones8[:, :], bias_bc[:, 0:HEADS],
            mybir.ActivationFunctionType.Identity, bias=1.0, scale=0.0,
        )


        def emit_proj_pair(xwy, qk, p):
            # chunks p (Q) and 4+p (K): exactly what head pair p's sims need
            for mm in (p, 4 + p):
                ps = proj_ps.tile([128, T], FP, tag="projy", name="proj")
                for k in range(XK):
                    nc.tensor.matmul(
                        ps[:, :],
                        lhsT=_r(wqk_sb[k][:, mm * 128:(mm + 1) * 128]),
                        rhs=_r(xwy[k][:, :]),
                        start=(k == 0),
                        stop=(k == XK - 1),
                    )
                t_ = qkpool.tile([128, T], BA, tag=f"qk{mm}", name=f"qk{mm}")
                nc.vector.tensor_copy(t_[:, :], ps[:, :])
                qk[mm] = t_

        def emit_sims(qk, m):
            us = ([], [])
            for j in range(4):
                psim = sim_ps.tile([128, 2 * T], FP, tag="sim", name="psim")
                for b in (0, 1):
                    lo, hi = b * 64, (b + 1) * 64
                    nc.tensor.matmul(
                        psim[:, b * T:(b + 1) * T],
                        lhsT=_r(qk[4 + m][lo:hi, j * 128:(j + 1) * 128]),
                        rhs=_r(qk[m][lo:hi, :]),
                        start=True,
                        stop=True,
                    )
                u = upool.tile([128, 2 * T], BA, tag=f"u{j}", name=f"u{j}")
                nc.scalar.activation(u[:, :], psim[:, :], Exp, scale=SCALE)
                us[0].append(u[:, 0:T])
                us[1].append(u[:, T:2 * T])
            return us

        def emit_repeat():
            # one self-contained pass over the 8 windows; no tile object
            # crosses the For_i back-edge, so the traced body is
            # iteration-invariant regardless of pool rotation phase.
            x_tiles = {0: load_x(0), 1: load_x(1)}
            win = {}

            def start_window(idx2):
                xwy2 = x_tiles.pop(idx2)
                qk2 = [None] * 8
                emit_proj_pair(xwy2, qk2, 0)
                win[idx2] = {"xwy": xwy2, "qk": qk2,
                             "uss": {0: emit_sims(qk2, 0)}}

            start_window(0)
            prev = None  # (y, o_sb) of the previous window
            for idx in range(NW):
                y = idx
                # prefetch the x slab two windows ahead (keeps loads ahead
                # of stores in the sync DMA queue)
                if idx + 2 < NW:
                    x_tiles[idx + 2] = load_x(idx + 2)
                st = win.pop(idx)
                xwy, qk, uss = st["xwy"], st["qk"], st["uss"]

                # remaining Q/K projection pairs (pair 0 was emitted by
                # start_window during the previous window, closing the ACT
                # bubble at the window boundary)
                for p in (1, 2, 3):
                    emit_proj_pair(xwy, qk, p)

                # ---- V projection, token-major with ones column per head --
                # v[t][token, h*65 + d], col h*65+64 == 1.0
                v = []
                for t in range(4):
                    ps = proj_ps.tile([128, INNER], FP, tag="projy", name="projv")
                    for k in range(XK):
                        nc.tensor.matmul(
                            ps[:, :],
                            lhsT=_r(xwy[k][:, t * 128:(t + 1) * 128]),
                            rhs=_r(wv_sb[k][:, :]),
                            start=(k == 0),
                            stop=(k == XK - 1),
                        )
                    t_ = vpool.tile([128, HEADS * 65], BA, tag=f"v{t}", name=f"v{t}")
                    dst = t_[:, :].rearrange("p (h e) -> p h e", e=65)
                    src = ps[:, :].rearrange("p (h e) -> p h e", e=64)
                    nc.vector.tensor_copy(dst[:, :, 0:64], src)
                    nc.vector.tensor_copy(t_[:, 64::65], ones8[:, :])
                    v.append(t_)

                # ---- attention, head pairs (2m, 2m+1), row-packed sim ----
                # The b=0 / b=1 sim matmuls use complementary 64-partition
                # row groups (auto tile_position from base_partition), so the
                # PE can run them concurrently.
                o_sb = [opool.tile([128, T], MR, tag=f"o{m}", name=f"o{m}") for m in range(4)]

                # sim+exp stage software-pipelined one pair ahead of the O
                # stage; at the last pair the NEXT window's preamble (proj
                # pair 0 + pair-0 sims) is emitted instead, so ACT's exp
                # stream never drains across in-repeat window boundaries
                #
                # normalize, batched: nc.vector.reciprocal streams its
                # free dim serially PER LANE, so a [1,T] recip is one-lane
                # and costs ~4.3us -- eight of them made DVE the kernel
                # bottleneck. Instead each O-group's po is drained to SBUF
                # with two cheap copies (payload -> oraw_b, denominator row
                # -> one lane of a shared [8,T] tile), freeing the po PSUM
                # WAR in <1us, and ONE [8,T] reciprocal at window end does
                # all 8 groups in the same 4.3us (8 lanes in parallel).
                # The broadcasts+multiplies run in the window tail,
                # overlapping the next window's projections; emit_y is
                # pushed to m==1 of the next window to give that tail two
                # full phases of slack before o_sb is consumed.
                s8 = rpool.tile([8, T], FP, tag="s8", name="s8")
                oraws = []
                for m in range(4):
                    if m < 3:
                        uss[m + 1] = emit_sims(qk, m + 1)
                    elif idx + 1 < NW:
                        start_window(idx + 1)
                    us = uss.pop(m)
                    # previous window's output projection: emitted inside
                    # pair 1 so the previous window's normalize tail
                    # (bcast+mult of its last O-groups) has two phases to
                    # retire before its o_sb is consumed
                    if m == 1 and prev is not None:
                        emit_y(*prev)
                        prev = None
                    for b in (0, 1):
                        h = 2 * m + b
                        k8 = 2 * m + b
                        lo, hi = b * 64, (b + 1) * 64
                        po = o_ps.tile([65, T], FP, tag="o_ps", name="po")
                        for j in range(4):
                            nc.tensor.matmul(
                                po[:, :],
                                lhsT=_r(v[j][:, h * 65:(h + 1) * 65]),
                                rhs=_r(us[b][j]),
                                start=(j == 0),
                                stop=(j == 3),
                            )
                        # one base-0 DVE copy drains payload AND the
                        # denominator row (DVE cannot write a single
                        # partition at offset>0); the s-row then hops to
                        # lane k8 of s8 via SBUF->SBUF DMA (no partition
                        # constraints, rides the idle SP ring)
                        oraw = orpool.tile([65, T], FP, tag=f"oraw{k8}", name=f"oraw{k8}")
                        nc.vector.tensor_copy(oraw[:, :], po[:, :])
                        nc.sync.dma_start(s8[k8:k8 + 1, :], oraw[64:65, :])
                        oraws.append((oraw, o_sb[m][lo:hi, :]))

                r8 = rpool.tile([8, T], FP, tag="r8", name="r8")
                nc.vector.reciprocal(r8[:, :], s8[:, :])
                # relocate the 8 reciprocal rows into one partition-0 row
                # (8 plain row-DMAs -- partition_broadcast requires base 0)
                # and do ONE [64, 8T] broadcast instead of 8 serial 1.5us
                # GPSIMD broadcasts; each multiply reads its column slice.
                rflat = rbpool.tile([1, 8 * T], FP, tag="rflat", name="rflat")
                for k8 in range(8):
                    nc.sync.dma_start(
                        rflat[:, k8 * T:(k8 + 1) * T], r8[k8:k8 + 1, :]
                    )
                rball = rbpool.tile([64, 8 * T], FP, tag="rball", name="rball")
                nc.gpsimd.partition_broadcast(rball[:, :], rflat[:, :])
                for k8, (oraw, o_dst) in enumerate(oraws):
                    nc.vector.tensor_tensor(
                        o_dst, oraw[0:64, :],
                        rball[:, k8 * T:(k8 + 1) * T], AluOpType.mult
                    )
                prev = (y, o_sb)

            emit_y(*prev)

        # repeats run inside a hardware loop: NEFF size is constant in
        # `repeat`, so the repeat-differencing harness measures true
        # steady-state execution, not NEFF ship/load overhead.
        if repeat > 1:
            with tc.For_i(0, repeat, 1):
                emit_repeat()
        else:
            emit_repeat()

    nc.compile()
    return nc


def _get_nc():
    key = ("nc", BF16_ATT)
    if key not in _CACHE:
        _CACHE[key] = _build()
    return _CACHE[key]


def _host_prep(x, wq, wkv, wo, bo):
    x = np.asarray(x, dtype=np.float32)
    wq = np.asarray(wq, dtype=np.float32)
    wkv = np.asarray(wkv, dtype=np.float32)
    wo = np.asarray(wo, dtype=np.float32)
    bo = np.asarray(bo, dtype=np.float32)

    wk = wkv[:INNER]
    wv = wkv[INNER:]
    wqkT = np.ascontiguousarray(np.concatenate([wq, wk], axis=0).T)  # (256, 1024)
    wvT = np.ascontiguousarray(wv.T)  # (256, 512)
    woT = np.ascontiguousarray(wo.T)  # (512, 256)
    bo2 = np.ascontiguousarray(bo.reshape(1, C))

    x0 = x[0]  # (256, 2, 128, 128)
    in_maps = []
    for c in range(N_CORES):
        xc = x0[:, :, c * WS:(c + 1) * WS, :]  # (256, 2, 16, 128)
        xc = xc.reshape(C, F, WS, NW, WS).transpose(0, 3, 1, 2, 4)  # (C, y, f, r, wl)
        xc = np.ascontiguousarray(xc.reshape(C, NPIX))
        in_maps.append({"xw": xc, "wqkT": wqkT, "wvT": wvT, "woT": woT, "bo": bo2})
    return in_maps


def _assemble(results):
    # per-core "out" is (NW, T, C) = (y, (f, r, wl), co); core c covers H rows
    # [16c, 16c+16).
    full = np.empty((1, C, F, H, W), dtype=np.float32)
    for c in range(N_CORES):
        oc = results[c]["out"]  # (8, 512, 256)
        oc = oc.reshape(NW, F, WS, WS, C).transpose(4, 1, 2, 0, 3)  # (C,f,r,y,wl)
        full[0, :, :, c * WS:(c + 1) * WS, :] = oc.reshape(C, F, WS, W)
    return full


def run(inputs, trace=False):
    nc = _get_nc()
    in_maps = _host_prep(**inputs)
    res = run_bass_kernel_spmd(
        nc, in_maps, core_ids=list(range(N_CORES)), trace=trace
    )
    out = _assemble(res.results)
    return out, res.exec_time_ns


def bench(inputs, iters=3):
    """Correct output + min wall-clock of the device execution (ns).

    No NTFF profiling hook exists in this environment, so the best available
    hardware number is wall time of the PJRT dispatch (includes axon tunnel
    overhead; min over iters approximates steady-state)."""
    import time

    nc = _get_nc()
    in_maps = _host_prep(**inputs)
    out = None
    best = None
    for _ in range(iters):
        t0 = time.perf_counter()
        res = run_bass_kernel_spmd(nc, in_maps, core_ids=list(range(N_CORES)))
        dt = (time.perf_counter() - t0) * 1e9
        best = dt if best is None else min(best, dt)
        out = _assemble(res.results)
    return out, best


def kernel(**inputs):
    out, _ = run(inputs, trace=False)
    return out


if __name__ == "__main__":
    rng = np.random.default_rng(0)
    ins = {
        "x": rng.standard_normal((1, C, F, H, W), dtype=np.float32),
        "wq": rng.standard_normal((INNER, C), dtype=np.float32) * C ** -0.5,
        "wkv": rng.standard_normal((2 * INNER, C), dtype=np.float32) * C ** -0.5,
        "wo": rng.standard_normal((C, INNER), dtype=np.float32) * INNER ** -0.5,
        "bo": np.zeros((C,), dtype=np.float32),
    }
    out = kernel(**ins)
    print(out.shape, out.dtype)



# revision 36
# speedup vs baseline: 1.2167x; 1.2167x over previous
"""Windowed 3D attention (nn_Attention3d) Trainium2 kernel, 8-core SPMD.

Sharding: the 8x8 grid of 16x16 spatial windows is split by row across the
8 NeuronCores. Each core processes a (C=256, F=2, 16, W=128) slab of x:
8 windows x 8 heads of independent 512-token attention, plus the QKV and
output projections for its slab. Host-side numpy does the window
permutation and weight transposes; cores see dense (C, tokens) panels.

Kernel structure (per window, fp32r matmuls, fp32 accumulation):
- Q/K projected inner-major (lhsT = wqk^T chunks), V projected token-major
  (lhsT = x panel), so attention needs no transposes anywhere:
  simT[j,i] = K^T-slice^T @ Q^T directly, with the two heads of an inner
  chunk row-packed into complementary 64-partition PE row groups writing
  one 2-bank PSUM tile.
- U = exp(simT/8) on ACT (one [128,1024] activation per head pair).
- O^T = (V|1)^T @ U accumulated over j; the appended ones column yields the
  softmax denominators s as PSUM row 64 for free. Normalization is batched
  per window: each O-group drains to SBUF with one [65,T] DVE copy (frees
  the po PSUM WAR in <1us), the 8 denominator rows hop to one [8,T] tile
  via SBUF->SBUF DMAs, and a single 8-lane DVE reciprocal covers the whole
  window (a [1,T] reciprocal is one-lane serial, ~4.3us each -- eight of
  them made DVE the kernel bottleneck). Broadcast (GPSIMD) + multiply (DVE)
  run in the window tail, overlapped with the next window's projections.
- Output projection consumes O^T chunks as lhsT; two i-tiles pack into one
  PSUM bank (from proj_ps, not o_ps, keeping the O stage's PSUM rotation
  free of y-stage consumers); bias added on DVE; strided output stores ride
  the ACT hardware-DGE queue so the SP queue never head-of-line blocks x
  prefetches behind stores.
- The output projection is software-pipelined one window behind (emitted at
  pair 1 so the previous normalize tail has two phases of slack); x panels
  prefetch two windows ahead on the sync DMA queue.
- Timing repeats run inside a tc.For_i hardware loop: NEFF size is constant
  in the repeat count, so repeat-differencing isolates true steady-state
  execution from NEFF-size-proportional dispatch overhead.

Measured (8 NeuronCores, axon, repeat-differenced): ~273 us per iteration,
rel err ~3e-4 vs the fp32 reference (fp32r matmul rounding).
"""

import os
import sys

sys.path.insert(0, "/opt/trn_rl_repo")
os.environ.setdefault("MYCRO_LOCAL_CACHE", "1")

import numpy as np
from contextlib import ExitStack

import concourse.bass as bass  # noqa: F401  (AP types)
import concourse.bacc as bacc
from concourse import mybir, tile
from concourse.alu_op_type import AluOpType
from concourse.bass_utils import run_bass_kernel_spmd

FP = mybir.dt.float32
HEADS = 8
DH = 64
WS = 16
C = 256
INNER = 512  # HEADS * DH
F = 2
H = 128
W = 128
NW = 8  # windows per core (one window-grid row)
T = F * WS * WS  # 512 tokens per window
NPIX = NW * T  # 4096 pixels per core slab
XK = 2  # C=256 -> 2 partition chunks of 128
N_CORES = 8
SCALE = DH ** -0.5
MR = mybir.dt.float32r  # rounded fp32: 1.5 vs 2.0 PE cyc/row; producers must emit it
BF16_ATT = os.environ.get("BF16_ATT", "1") == "1"  # bf16 sim/O matmul operands
BA = mybir.dt.bfloat16 if BF16_ATT else MR


def _r(ap):
    return ap

_CACHE = {}


def _build(repeat=1):
    nc = bacc.Bacc("TRN2", target_bir_lowering=False, debug=False)

    xw = nc.dram_tensor("xw", [C, NPIX], MR, kind="ExternalInput").ap()
    wqkT = nc.dram_tensor("wqkT", [C, 2 * INNER], MR, kind="ExternalInput").ap()
    wvT = nc.dram_tensor("wvT", [C, INNER], MR, kind="ExternalInput").ap()
    woT = nc.dram_tensor("woT", [INNER, C], MR, kind="ExternalInput").ap()
    bo = nc.dram_tensor("bo", [1, C], FP, kind="ExternalInput").ap()
    out = nc.dram_tensor("out", [NW, T, C], FP, kind="ExternalOutput").ap()

    Exp = mybir.ActivationFunctionType.Exp

    with tile.TileContext(nc) as tc, ExitStack() as ctx:
        def pool(name, bufs, space="SBUF"):
            return ctx.enter_context(tc.tile_pool(name=name, bufs=bufs, space=space))

        consts = pool("consts", 1)
        xpool = pool("x", 3)
        qkpool = pool("qk", 2)
        vpool = pool("v", 2)
        upool = pool("u", 2)
        rpool = pool("r", 2)
        orpool = pool("oraw", 2)
        opool = pool("o", 2)
        ypool = pool("y", 2)

        proj_ps = pool("proj_ps", 2, space="PSUM")
        sim_ps = pool("sim_ps", 2, space="PSUM")
        o_ps = pool("o_ps", 2, space="PSUM")
        y_ps = o_ps  # y tiles borrow o_ps slots; proj pool stays open for the next window

        def load_x(yy):
            ts = []
            for k in range(XK):
                t_ = xpool.tile([128, T], MR, tag=f"x{k}", name=f"x{k}")
                nc.sync.dma_start(t_[:, :], xw[k * 128:(k + 1) * 128, yy * T:(yy + 1) * T])
                ts.append(t_)
            return ts

        def emit_y(yy, o_sb):
            # output projection + bias (pipelined one window behind so the
            # PE stream interleaves it with the NEXT window's projections).
            # Two i-tiles pack into one PSUM bank (N=256 halves). py comes
            # from proj_ps, NOT o_ps: sharing o_ps with po made the next
            # window's O matmuls WAR-wait on the y-stage DVE consumer
            # (-84us/repeat). The store rides the ACT hardware DGE queue so
            # the SP queue never head-of-line blocks x prefetches behind a
            # store whose ysb data isn't ready yet (-75us/repeat).
            for it2 in range(2):
                py = proj_ps.tile([128, 2 * C], FP, tag="projy", name="py")
                for half in range(2):
                    it = 2 * it2 + half
                    for m in range(4):
                        nc.tensor.matmul(
                            py[:, half * C:(half + 1) * C],
                            lhsT=_r(o_sb[m][:, it * 128:(it + 1) * 128]),
                            rhs=_r(wo_sb[m][:, :]),
                            start=(m == 0),
                            stop=(m == 3),
                        )
                ysb = ypool.tile([128, 2 * C], FP, tag="y", name="ysb")
                nc.vector.tensor_tensor(
                    ysb[:, :], py[:, :], bias2_bc[:, :], AluOpType.add
                )
                dst = out[yy, it2 * 256:(it2 + 1) * 256, :].rearrange(
                    "(h p) c -> p h c", h=2
                )
                nc.scalar.dma_start(dst, ysb[:, :].rearrange("p (h c) -> p h c", h=2))

        # pin the exp activation-table set while the initial DMAs stream,
        # so the ~2.7us ACT_TABLE_LOAD is off window 0's critical path
        warm = consts.tile([128, 8], FP, tag="warm", name="warm")
        nc.vector.memset(warm[:, :], 0.0)
        nc.scalar.activation(warm[:, :], warm[:, :], Exp, scale=1.0)
        # HAM warmup: dummy matmuls during the initial DMA wait so the PE
        # clock-gate is at 2.4GHz (not cold 1.2) when window 0 starts
        wmm = consts.tile([128, 640], MR, tag="wmm", name="wmm")
        nc.vector.memset(wmm[:, :].bitcast(FP), 0.0)
        for _ in range(26):
            wps = proj_ps.tile([128, T], FP, tag="projy", name="wps")
            nc.tensor.matmul(
                wps[:, :], lhsT=wmm[:, 0:128], rhs=wmm[:, 128:640],
                start=True, stop=True,
            )

        # ---- weights / constants (loaded once) ----
        wqk_sb = []
        wv_sb = []
        wo_sb = []
        for k in range(XK):
            t_ = consts.tile([128, 2 * INNER], MR, tag=f"wqk{k}", name=f"wqk{k}")
            nc.sync.dma_start(t_[:, :], wqkT[k * 128:(k + 1) * 128, :])
            wqk_sb.append(t_)
            t_ = consts.tile([128, INNER], MR, tag=f"wv{k}", name=f"wv{k}")
            nc.sync.dma_start(t_[:, :], wvT[k * 128:(k + 1) * 128, :])
            wv_sb.append(t_)
        for m in range(4):
            t_ = consts.tile([128, C], MR, tag=f"wo{m}", name=f"wo{m}")
            nc.sync.dma_start(t_[:, :], woT[m * 128:(m + 1) * 128, :])
            wo_sb.append(t_)
        bo_sb = consts.tile([1, C], FP, tag="bo")
        nc.sync.dma_start(bo_sb[:, :], bo[:, :])
        bias_bc = consts.tile([128, C], FP, tag="bias_bc")
        nc.gpsimd.partition_broadcast(bias_bc[:, :], bo_sb[:, :])
        bias2_bc = consts.tile([128, 2 * C], FP, tag="bias2_bc")
        nc.vector.tensor_copy(bias2_bc[:, 0:C], bias_bc[:, :])
        nc.vector.tensor_copy(bias2_bc[:, C:2 * C], bias_bc[:, :])
        ones8 = consts.tile([128, HEADS], BA, tag="ones8")
        nc.scalar.activation(
            ones8[:, :], bias_bc[:, 0:HEADS],
            mybir.ActivationFunctionType.Identity, bias=1.0, scale=0.0,
        )


        def emit_proj_pair(xwy, qk, p):
            # chunks p (Q) and 4+p (K): exactly what head pair p's sims need
            for mm in (p, 4 + p):
                ps = proj_ps.tile([128, T], FP, tag="projy", name="proj")
                for k in range(XK):
                    nc.tensor.matmul(
                        ps[:, :],
                        lhsT=_r(wqk_sb[k][:, mm * 128:(mm + 1) * 128]),
                        rhs=_r(xwy[k][:, :]),
                        start=(k == 0),
                        stop=(k == XK - 1),
                    )
                t_ = qkpool.tile([128, T], BA, tag=f"qk{mm}", name=f"qk{mm}")
                nc.vector.tensor_copy(t_[:, :], ps[:, :])
                qk[mm] = t_

        def emit_sims(qk, m):
            us = ([], [])
            for j in range(4):
                psim = sim_ps.tile([128, 2 * T], FP, tag="sim", name="psim")
                for b in (0, 1):
                    lo, hi = b * 64, (b + 1) * 64
                    nc.tensor.matmul(
                        psim[:, b * T:(b + 1) * T],
                        lhsT=_r(qk[4 + m][lo:hi, j * 128:(j + 1) * 128]),
                        rhs=_r(qk[m][lo:hi, :]),
                        start=True,
                        stop=True,
                    )
                u = upool.tile([128, 2 * T], BA, tag=f"u{j}", name=f"u{j}")
                nc.scalar.activation(u[:, :], psim[:, :], Exp, scale=SCALE)
                us[0].append(u[:, 0:T])
                us[1].append(u[:, T:2 * T])
            return us

        def emit_repeat():
            # one self-contained pass over the 8 windows; no tile object
            # crosses the For_i back-edge, so the traced body is
            # iteration-invariant regardless of pool rotation phase.
            x_tiles = {0: load_x(0), 1: load_x(1)}
            win = {}

            def start_window(idx2):
                xwy2 = x_tiles.pop(idx2)
                qk2 = [None] * 8
                emit_proj_pair(xwy2, qk2, 0)
                win[idx2] = {"xwy": xwy2, "qk": qk2,
                             "uss": {0: emit_sims(qk2, 0)}}

            start_window(0)
            prev = None  # (y, o_sb) of the previous window
            for idx in range(NW):
                y = idx
                # prefetch the x slab two windows ahead (keeps loads ahead
                # of stores in the sync DMA queue)
                if idx + 2 < NW:
                    x_tiles[idx + 2] = load_x(idx + 2)
                st = win.pop(idx)
                xwy, qk, uss = st["xwy"], st["qk"], st["uss"]

                # remaining Q/K projection pairs (pair 0 was emitted by
                # start_window during the previous window, closing the ACT
                # bubble at the window boundary)
                for p in (1, 2, 3):
                    emit_proj_pair(xwy, qk, p)

                # ---- V projection, token-major with ones column per head --
                # v[t][token, h*65 + d], col h*65+64 == 1.0
                v = []
                for t in range(4):
                    ps = proj_ps.tile([128, INNER], FP, tag="projy", name="projv")
                    for k in range(XK):
                        nc.tensor.matmul(
                            ps[:, :],
                            lhsT=_r(xwy[k][:, t * 128:(t + 1) * 128]),
                            rhs=_r(wv_sb[k][:, :]),
                            start=(k == 0),
                            stop=(k == XK - 1),
                        )
                    t_ = vpool.tile([128, HEADS * 65], BA, tag=f"v{t}", name=f"v{t}")
                    dst = t_[:, :].rearrange("p (h e) -> p h e", e=65)
                    src = ps[:, :].rearrange("p (h e) -> p h e", e=64)
                    nc.vector.tensor_copy(dst[:, :, 0:64], src)
                    nc.vector.tensor_copy(t_[:, 64::65], ones8[:, :])
                    v.append(t_)

                # ---- attention, head pairs (2m, 2m+1), row-packed sim ----
                # The b=0 / b=1 sim matmuls use complementary 64-partition
                # row groups (auto tile_position from base_partition), so the
                # PE can run them concurrently.
                o_sb = [opool.tile([128, T], MR, tag=f"o{m}", name=f"o{m}") for m in range(4)]

                # sim+exp stage software-pipelined one pair ahead of the O
                # stage; at the last pair the NEXT window's preamble (proj
                # pair 0 + pair-0 sims) is emitted instead, so ACT's exp
                # stream never drains across in-repeat window boundaries
                #
                # normalize, batched: nc.vector.reciprocal streams its
                # free dim serially PER LANE, so a [1,T] recip is one-lane
                # and costs ~4.3us -- eight of them made DVE the kernel
                # bottleneck. Instead each O-group's po is drained to SBUF
                # with two cheap copies (payload -> oraw_b, denominator row
                # -> one lane of a shared [8,T] tile), freeing the po PSUM
                # WAR in <1us, and ONE [8,T] reciprocal at window end does
                # all 8 groups in the same 4.3us (8 lanes in parallel).
                # The broadcasts+multiplies run in the window tail,
                # overlapping the next window's projections; emit_y is
                # pushed to m==1 of the next window to give that tail two
                # full phases of slack before o_sb is consumed.
                s8 = rpool.tile([8, T], FP, tag="s8", name="s8")
                oraws = []
                for m in range(4):
                    if m < 3:
                        uss[m + 1] = emit_sims(qk, m + 1)
                    elif idx + 1 < NW:
                        start_window(idx + 1)
                    us = uss.pop(m)
                    # previous window's output projection: emitted inside
                    # pair 1 so the previous window's normalize tail
                    # (bcast+mult of its last O-groups) has two phases to
                    # retire before its o_sb is consumed
                    if m == 1 and prev is not None:
                        emit_y(*prev)
                        prev = None
                    for b in (0, 1):
                        h = 2 * m + b
                        k8 = 2 * m + b
                        lo, hi = b * 64, (b + 1) * 64
                        po = o_ps.tile([65, T], FP, tag="o_ps", name="po")
                        for j in range(4):
                            nc.tensor.matmul(
                                po[:, :],
                                lhsT=_r(v[j][:, h * 65:(h + 1) * 65]),
                                rhs=_r(us[b][j]),
                                start=(j == 0),
                                stop=(j == 3),
                            )
                        # one base-0 DVE copy drains payload AND the
                        # denominator row (DVE cannot write a single
                        # partition at offset>0); the s-row then hops to
                        # lane k8 of s8 via SBUF->SBUF DMA (no partition
                        # constraints, rides the idle SP ring)
                        oraw = orpool.tile([65, T], FP, tag=f"oraw{k8}", name=f"oraw{k8}")
                        nc.vector.tensor_copy(oraw[:, :], po[:, :])
                        nc.sync.dma_start(s8[k8:k8 + 1, :], oraw[64:65, :])
                        oraws.append((oraw, o_sb[m][lo:hi, :]))

                r8 = rpool.tile([8, T], FP, tag="r8", name="r8")
                nc.vector.reciprocal(r8[:, :], s8[:, :])
                rbs = []
                for k8 in range(8):
                    # partition_broadcast's ucode requires its input at
                    # partition 0: relocate row k8 there with a tiny DMA
                    rone = rpool.tile([1, T], FP, tag=f"rone{k8 % 4}", name="rone")
                    nc.sync.dma_start(rone[:, :], r8[k8:k8 + 1, :])
                    rb = rpool.tile([64, T], FP, tag=f"rb{k8 % 4}", name="rb")
                    nc.gpsimd.partition_broadcast(rb[:, :], rone[:, :])
                    rbs.append(rb)
                for k8, (oraw, o_dst) in enumerate(oraws):
                    nc.vector.tensor_tensor(
                        o_dst, oraw[0:64, :], rbs[k8][:, :], AluOpType.mult
                    )
                prev = (y, o_sb)

            emit_y(*prev)

        # repeats run inside a hardware loop: NEFF size is constant in
        # `repeat`, so the repeat-differencing harness measures true
        # steady-state execution, not NEFF ship/load overhead.
        if repeat > 1:
            with tc.For_i(0, repeat, 1):
                emit_repeat()
        else:
            emit_repeat()

    nc.compile()
    return nc


def _get_nc():
    key = ("nc", BF16_ATT)
    if key not in _CACHE:
        _CACHE[key] = _build()
    return _CACHE[key]


def _host_prep(x, wq, wkv, wo, bo):
    x = np.asarray(x, dtype=np.float32)
    wq = np.asarray(wq, dtype=np.float32)
    wkv = np.asarray(wkv, dtype=np.float32)
    wo = np.asarray(wo, dtype=np.float32)
    bo = np.asarray(bo, dtype=np.float32)

    wk = wkv[:INNER]
    wv = wkv[INNER:]
    wqkT = np.ascontiguousarray(np.concatenate([wq, wk], axis=0).T)  # (256, 1024)
    wvT = np.ascontiguousarray(wv.T)  # (256, 512)
    woT = np.ascontiguousarray(wo.T)  # (512, 256)
    bo2 = np.ascontiguousarray(bo.reshape(1, C))

    x0 = x[0]  # (256, 2, 128, 128)
    in_maps = []
    for c in range(N_CORES):
        xc = x0[:, :, c * WS:(c + 1) * WS, :]  # (256, 2, 16, 128)
        xc = xc.reshape(C, F, WS, NW, WS).transpose(0, 3, 1, 2, 4)  # (C, y, f, r, wl)
        xc = np.ascontiguousarray(xc.reshape(C, NPIX))
        in_maps.append({"xw": xc, "wqkT": wqkT, "wvT": wvT, "woT": woT, "bo": bo2})
    return in_maps


def _assemble(results):
    # per-core "out" is (NW, T, C) = (y, (f, r, wl), co); core c covers H rows
    # [16c, 16c+16).
    full = np.empty((1, C, F, H, W), dtype=np.float32)
    for c in range(N_CORES):
        oc = results[c]["out"]  # (8, 512, 256)
        oc = oc.reshape(NW, F, WS, WS, C).transpose(4, 1, 2, 0, 3)  # (C,f,r,y,wl)
        full[0, :, :, c * WS:(c + 1) * WS, :] = oc.reshape(C, F, WS, W)
    return full


def run(inputs, trace=False):
    nc = _get_nc()
    in_maps = _host_prep(**inputs)
    res = run_bass_kernel_spmd(
        nc, in_maps, core_ids=list(range(N_CORES)), trace=trace
    )
    out = _assemble(res.results)
    return out, res.exec_time_ns


def bench(inputs, iters=3):
    """Correct output + min wall-clock of the device execution (ns).

    No NTFF profiling hook exists in this environment, so the best available
    hardware number is wall time of the PJRT dispatch (includes axon tunnel
    overhead; min over iters approximates steady-state)."""
    import time

    nc = _get_nc()
    in_maps = _host_prep(**inputs)
    out = None
    best = None
    for _ in range(iters):
        t0 = time.perf_counter()
        res = run_bass_kernel_spmd(nc, in_maps, core_ids=list(range(N_CORES)))
        dt = (time.perf_counter() - t0) * 1e9
        best = dt if best is None else min(best, dt)
        out = _assemble(res.results)
    return out, best


def kernel(**inputs):
    out, _ = run(inputs, trace=False)
    return out


if __name__ == "__main__":
    rng = np.random.default_rng(0)
    ins = {
        "x": rng.standard_normal((1, C, F, H, W), dtype=np.float32),
        "wq": rng.standard_normal((INNER, C), dtype=np.float32) * C ** -0.5,
        "wkv": rng.standard_normal((2 * INNER, C), dtype=np.float32) * C ** -0.5,
        "wo": rng.standard_normal((C, INNER), dtype=np.float32) * INNER ** -0.5,
        "bo": np.zeros((C,), dtype=np.float32),
    }
    out = kernel(**ins)
    print(out.shape, out.dtype)

